# revision 1
# baseline (speedup 1.0000x reference)
"""DeepseekV2 MoE (T=2048, H=2048, E=16 experts, top-6, I=1408, shared IS=2816)
on 8 Trainium2 NeuronCores.

Strategy (expert-parallel per the sharding hint):
  - Host: gate softmax/top-6 (numpy replica of the reference; the top-6/7
    logit gap is ~7e-5 so the selection is rounding-robust), per-expert token
    gather, weight transpose/packing into DMA-friendly layouts, and the final
    scatter/combine (0.05% of the FLOPs).
  - Device (SPMD, 8 cores): core i owns routed experts 2i and 2i+1 (dense
    SwiGLU over a fixed capacity C=896 of gathered tokens, combine weights
    applied on-chip) plus 1/8 of the shared expert (tensor-parallel over the
    intermediate dim, 352 padded to 384). All matmuls run in float32r
    (FP22 truncation — full PE throughput at free-dim >= 256).
  - No collectives: per-core outputs are disjoint (routed) or partial sums
    (shared) that the host adds.
"""

import os
import numpy as np

import concourse.bass as bass
import concourse.mybir as mybir
import concourse.tile as tile
from concourse.bass_utils import run_bass_kernel_spmd

F32 = mybir.dt.float32
F32R = mybir.dt.float32r
AF = mybir.ActivationFunctionType

# problem dims (hardcoded per spec)
T, H, I, E, TOP_K = 2048, 2048, 1408, 16, 6
FF = 2 * I              # 2816
IS = 2 * I              # shared intermediate
N_CORES = 8
C = 896                 # per-expert token capacity (binomial mean 768, sd ~22;
                        # seed-0 max count is 818; overflow falls back to host)
ISP = 384               # per-core shared-intermediate slice, 352 padded to 384

HC = H // 128           # 16 H chunks (contraction for mm1)
IC = I // 128           # 11 I chunks (contraction for mm2)
HB = H // 512           # 4 output H blocks
KS = ISP // 128         # 3 shared-intermediate chunks
TBP = T // 1024         # 2 shared token super-blocks (1024 tokens each)


def _mm_blocks(width):
    """Moving-dim blocks of 512 with a >=256 tail (fp32r full rate needs >=256)."""
    out, off = [], 0
    while off < width:
        w = min(512, width - off)
        assert w >= 256
        out.append((off, w))
        off += w
    return out


def _split_excess_waits(nc, cap=1):
    """This container's walrus accepts at most one semaphore wait per
    instruction; move excess waits onto inserted same-engine NOPs."""
    for bb in nc.main_func.blocks:
        new_list = []
        for ins in bb.instructions:
            si = getattr(ins, "sync_info", None)
            waits = list(si.on_wait) if (si is not None and si.on_wait) else []
            if len(waits) > cap:
                excess, keep = waits[:-cap], waits[-cap:]
                si.on_wait = keep
                for i in range(0, len(excess), cap):
                    nop = mybir.InstNoOp(
                        name=f"I-waitsplit-{nc.next_id()}",
                        engine=ins.engine,
                        ins=[],
                        outs=[],
                        sync_info=mybir.SyncInfo(
                            on_update=[], on_wait=excess[i : i + cap]
                        ),
                        bass_nofuse=True,
                    )
                    nc.register_instruction(nop, overwrite=True)
                    new_list.append(nop)
            new_list.append(ins)
        bb.instructions = new_list


def build_nc(cap: int):
    """Build the per-core Bass program for token capacity `cap` (mult of 512)."""
    cc_n = cap // 128
    nb_n = cap // 512
    nc = bass.Bass()

    # --- DRAM parameters (packed layouts; partition dim = 128 first) ---
    # x.T gathered per owned expert: [slot][128p(H in), HC, cap]
    xt_d = [
        nc.declare_dram_parameter(f"xt{s}", [128, HC, cap], F32R, isOutput=False)
        for s in range(2)
    ]
    # w13[e].T blocks, order g0,u0,g1,u1,...: [2*IC][128p(H in), HC, 128]
    w13_d = [
        nc.declare_dram_parameter(f"w13_{s}", [2 * IC, 128, HC, 128], F32R, isOutput=False)
        for s in range(2)
    ]
    # w2[e].T blocks per output H block: [HB][128p(I in), IC, 512]
    w2_d = [
        nc.declare_dram_parameter(f"w2_{s}", [HB, 128, IC, 512], F32R, isOutput=False)
        for s in range(2)
    ]
    # x.T full (shared expert), token super-blocks: [TBP][128p(H in), HC, 1024]
    xts_d = nc.declare_dram_parameter("xts", [TBP, 128, HC, 1024], F32R, isOutput=False)
    # shared w13 slice blocks (g0,u0,g1,u1,g2,u2): [6][128p(H in), HC, 128]
    sw13_d = nc.declare_dram_parameter("sw13", [2 * KS, 128, HC, 128], F32R, isOutput=False)
    # shared w2 slice blocks: [HB][128p(ISP in), KS, 512]
    sw2_d = nc.declare_dram_parameter("sw2", [HB, 128, KS, 512], F32R, isOutput=False)
    # combine weights: [128, 2 * cc_n] (col s*cc_n+cc -> tokens cc*128..+128 of slot s)
    c_d = nc.declare_dram_parameter("cvec", [128, 2 * cc_n], F32, isOutput=False)

    yout_d = [
        nc.declare_dram_parameter(f"yout{s}", [cap, H], F32, isOutput=True)
        for s in range(2)
    ]
    ys_d = nc.declare_dram_parameter("ys", [T, H], F32, isOutput=True)

    with tile.TileContext(nc) as tc:
        with (
            tc.tile_pool(name="xt", bufs=1) as p_xt,
            tc.tile_pool(name="w13", bufs=3) as p_w13,
            tc.tile_pool(name="w2", bufs=2) as p_w2,
            tc.tile_pool(name="tmp", bufs=3) as p_tmp,
            tc.tile_pool(name="aT", bufs=1) as p_aT,
            tc.tile_pool(name="y", bufs=4) as p_y,
            tc.tile_pool(name="c", bufs=1) as p_c,
            tc.tile_pool(name="ps", bufs=8, space="PSUM") as p_ps,
        ):
            c_sb = p_c.tile([128, 2 * cc_n], F32)
            nc.sync.dma_start(out=c_sb[:], in_=c_d[:])

            def load_xt(dram_src, width):
                """Load an x.T block with per-H-chunk strip DMAs, ordered so
                the first 512-wide block (what the first PSUM accumulation
                group consumes) lands first."""
                t = p_xt.tile([128, HC, width], F32R, tag="xt")
                for off, w in _mm_blocks(width):
                    for hc in range(HC):
                        nc.sync.dma_start(
                            out=t[:, hc, off:off + w],
                            in_=dram_src[:, hc, off:off + w],
                        )
                return t

            def swiglu_mm1(xt_sb, w13_src, n_i, aT_sb, width):
                """mm1 + SiLU*u for one weight set.
                xt_sb: [128, HC, width]; w13_src: DRAM [2*n_i, 128, HC, 128];
                aT_sb: [128, n_i, width] destination (f32r)."""
                for i in range(n_i):
                    wg = p_w13.tile([128, HC, 128], F32R, tag="w13")
                    nc.sync.dma_start(out=wg[:], in_=w13_src[2 * i])
                    wu = p_w13.tile([128, HC, 128], F32R, tag="w13")
                    nc.sync.dma_start(out=wu[:], in_=w13_src[2 * i + 1])
                    for off, w in _mm_blocks(width):
                        col = slice(off, off + w)
                        ps_g = p_ps.tile([128, 512], F32, tag="ps")
                        ps_u = p_ps.tile([128, 512], F32, tag="ps")
                        for hc in range(HC):
                            nc.tensor.matmul(
                                ps_g[:, :w], wg[:, hc, :], xt_sb[:, hc, col],
                                start=(hc == 0), stop=(hc == HC - 1),
                            )
                        for hc in range(HC):
                            nc.tensor.matmul(
                                ps_u[:, :w], wu[:, hc, :], xt_sb[:, hc, col],
                                start=(hc == 0), stop=(hc == HC - 1),
                            )
                        tmp = p_tmp.tile([128, 512], F32, tag="tmp")
                        nc.scalar.activation(
                            out=tmp[:, :w], in_=ps_g[:, :w], func=AF.Silu
                        )
                        nc.vector.tensor_mul(
                            out=aT_sb[:, i, col], in0=tmp[:, :w], in1=ps_u[:, :w]
                        )

            # One shared-expert token super-block (1024 tokens, 1/8 TP slice)
            def shared_phase(tbp):
                xts_sb = load_xt(xts_d[tbp], 1024)

                aTs = p_aT.tile([128, KS, 1024], F32R, tag="aT")
                swiglu_mm1(xts_sb, sw13_d, KS, aTs, 1024)

                for hb in range(HB):
                    sw2b = p_w2.tile([128, KS, 512], F32R, tag="w2")
                    nc.sync.dma_start(out=sw2b[:], in_=sw2_d[hb])
                    for tc_ in range(8):
                        ps_y = p_ps.tile([128, 512], F32, tag="ps")
                        for k in range(KS):
                            nc.tensor.matmul(
                                ps_y[:],
                                aTs[:, k, tc_ * 128:(tc_ + 1) * 128],
                                sw2b[:, k, :],
                                start=(k == 0), stop=(k == KS - 1),
                            )
                        y_sb = p_y.tile([128, 512], F32, tag="y")
                        nc.vector.tensor_copy(y_sb[:], ps_y[:])
                        row0 = tbp * 1024 + tc_ * 128
                        nc.gpsimd.dma_start(
                            out=ys_d[row0:row0 + 128, hb * 512:(hb + 1) * 512],
                            in_=y_sb[:],
                        )

            # One routed expert (dense over the capacity token set)
            def expert_phase(s):
                xt_sb = load_xt(xt_d[s], cap)

                aT = p_aT.tile([128, IC, cap], F32R, tag="aT")
                swiglu_mm1(xt_sb, w13_d[s], IC, aT, cap)

                # mm2: y[c,h] = sum_i a[c,i] * w2T[i,h], c-scaled on evac
                for hb in range(HB):
                    w2b = p_w2.tile([128, IC, 512], F32R, tag="w2")
                    nc.sync.dma_start(out=w2b[:], in_=w2_d[s][hb])
                    for cc in range(cc_n):
                        ps_y = p_ps.tile([128, 512], F32, tag="ps")
                        for ic in range(IC):
                            nc.tensor.matmul(
                                ps_y[:],
                                aT[:, ic, cc * 128:(cc + 1) * 128],
                                w2b[:, ic, :],
                                start=(ic == 0), stop=(ic == IC - 1),
                            )
                        y_sb = p_y.tile([128, 512], F32, tag="y")
                        nc.vector.tensor_scalar_mul(
                            y_sb[:], ps_y[:], c_sb[:, s * cc_n + cc : s * cc_n + cc + 1]
                        )
                        nc.gpsimd.dma_start(
                            out=yout_d[s][cc * 128:(cc + 1) * 128,
                                          hb * 512:(hb + 1) * 512],
                            in_=y_sb[:],
                        )

            # Interleave: each phase's loads prefetch under the previous
            # phase's compute instead of colliding at phase boundaries.
            shared_phase(0)
            expert_phase(0)
            shared_phase(1)
            expert_phase(1)

    _split_excess_waits(nc, cap=1)
    return nc


# ------------------------- host side -------------------------

def _gate_combine(x, gate_w):
    """Replica of the reference gate in pure numpy (f32). The top-6 selection
    is what must match the reference exactly; the smallest rank-6/rank-7 logit
    gap over the 2048 tokens is ~7e-5 while cross-implementation f32 rounding
    differences are ~1e-6, so the selection is identical. Tie-break on exact
    equality follows lax.top_k (lowest index wins)."""
    z = (x @ gate_w.T).astype(np.float32)                 # [T, E] logits
    z64 = z.astype(np.float64)
    m = z64.max(-1, keepdims=True)
    ez = np.exp(z64 - m)
    scores = (ez / ez.sum(-1, keepdims=True)).astype(np.float32)
    # top-6 per token, ties broken by lowest expert index (argsort on
    # (-score, idx) via stable mergesort on -score)
    order = np.argsort(-scores, axis=-1, kind="stable")[:, :TOP_K]
    topk_w = np.take_along_axis(scores, order, axis=-1)
    topk_w = topk_w / (topk_w.sum(-1, keepdims=True) + 1e-20)
    combine = np.zeros((x.shape[0], E), np.float32)
    np.put_along_axis(combine, order, topk_w, axis=-1)
    return combine


def _pack_w13(w13e):
    """w13[e] [FF, H] -> [2*IC, 128, HC, 128] with block order g0,u0,g1,u1,..."""
    # w13e.T is [H, FF]; block j covers FF cols j*128..(j+1)*128
    # reshape w13e [FF, H] = [2*IC jb, 128 f, HC o, 128 p] -> [jb, p, o, f]
    a = np.ascontiguousarray(
        w13e.reshape(2 * IC, 128, HC, 128).transpose(0, 3, 2, 1)
    )
    order = np.empty(2 * IC, np.int64)
    order[0::2] = np.arange(IC)           # gate chunks 0..10
    order[1::2] = np.arange(IC) + IC      # up chunks 11..21
    return np.ascontiguousarray(a[order])


def _pack_w2(w2e):
    """w2[e] [H, I] -> [HB, 128, IC, 512]: w2T[i, h] with i=o*128+p, h=hb*512+f."""
    return np.ascontiguousarray(
        w2e.reshape(HB, 512, IC, 128).transpose(0, 3, 2, 1)
    )


def _pack_xT(xT, width):
    """xT [H, n*width] -> [n, 128, HC, width]"""
    n = xT.shape[1] // width
    return np.ascontiguousarray(
        xT.reshape(HC, 128, n, width).transpose(2, 1, 0, 3)
    )


def _host_moe(x, combine, w13, w2, sw13, sw2):
    """Exact numpy fallback (only used on absurd routing imbalance)."""

    def silu(v):
        return v / (1.0 + np.exp(-v))

    out = np.zeros((T, H), np.float32)
    for e in range(E):
        gu = x @ w13[e].T
        a = silu(gu[:, :I]) * gu[:, I:]
        out += combine[:, e:e + 1] * (a @ w2[e].T)
    gu = x @ sw13.T
    a = silu(gu[:, :IS]) * gu[:, IS:]
    out += a @ sw2.T
    return out


_NC_CACHE = {}

LAST_EXEC_TIME_NS = None
LAST_TRACE = None


def _install_ntff_hook():
    """Bridge the missing ``antenv.axon_hooks`` module so trace=True works
    in this container (used by test.py only; harmless if already present)."""
    import sys, types

    try:
        from antenv.axon_hooks import get_axon_ntff_profile_hook  # noqa: F401
        return
    except ImportError:
        pass
    import antenv  # noqa: F401
    import trn_agent_boot.trn_boot as tb

    mod = types.ModuleType("antenv.axon_hooks")
    _h = [None]
    mod.set_axon_ntff_profile_hook = lambda h: _h.__setitem__(0, h)
    mod.get_axon_ntff_profile_hook = lambda: _h[0]
    sys.modules["antenv.axon_hooks"] = mod
    mod.set_axon_ntff_profile_hook(
        tb._ntff_profile_via_ctypes("/opt/axon/libaxon_pjrt.so")
    )


def kernel(hidden_states, gate_w, w13, w2, sw13, sw2):
    hidden_states = np.asarray(hidden_states)
    x = np.ascontiguousarray(hidden_states.reshape(T, H), dtype=np.float32)
    gate_w = np.asarray(gate_w, dtype=np.float32)
    w13 = np.asarray(w13, dtype=np.float32)
    w2 = np.asarray(w2, dtype=np.float32)
    sw13 = np.asarray(sw13, dtype=np.float32)
    sw2 = np.asarray(sw2, dtype=np.float32)

    combine = _gate_combine(x, gate_w)          # [T, E]

    ids = [np.nonzero(combine[:, e] > 0)[0] for e in range(E)]
    max_n = max(len(i) for i in ids)
    if max_n > C:
        # Essentially impossible for randn-style inputs (needs an 11-sigma
        # routing imbalance); fall back to an exact host computation.
        return _host_moe(x, combine, w13, w2, sw13, sw2).reshape(
            hidden_states.shape
        )
    cap = C

    if cap not in _NC_CACHE:
        _NC_CACHE[cap] = build_nc(cap)
    nc = _NC_CACHE[cap]

    xT = np.ascontiguousarray(x.T)              # [H, T]
    xts_p = _pack_xT(xT, 1024)                  # [TBP, 128, HC, 1024]

    in_maps = []
    for core in range(N_CORES):
        m = {"xts": xts_p}
        cvec = np.zeros((128, 2 * (cap // 128)), np.float32)
        for s in range(2):
            e = 2 * core + s
            tok = ids[e]
            xt_e = np.zeros((H, cap), np.float32)
            xt_e[:, : len(tok)] = xT[:, tok]
            m[f"xt{s}"] = _pack_xT(xt_e, cap)[0]
            m[f"w13_{s}"] = _pack_w13(w13[e])
            m[f"w2_{s}"] = _pack_w2(w2[e])
            cw = np.zeros(cap, np.float32)
            cw[: len(tok)] = combine[tok, e]
            cvec[:, s * (cap // 128):(s + 1) * (cap // 128)] = (
                cw.reshape(cap // 128, 128).T
            )
        m["cvec"] = cvec

        # shared expert slice (352 rows padded to ISP=384)
        lo, hi = core * 352, (core + 1) * 352
        gsl = np.zeros((ISP, H), np.float32)
        usl = np.zeros((ISP, H), np.float32)
        gsl[:352] = sw13[lo:hi]
        usl[:352] = sw13[IS + lo: IS + hi]
        # block q=2k -> gate chunk k; q=2k+1 -> up chunk k; each [128p(H), HC, 128f]
        gb = gsl.reshape(KS, 128, HC, 128).transpose(0, 3, 2, 1)
        ub = usl.reshape(KS, 128, HC, 128).transpose(0, 3, 2, 1)
        sw13_p = np.empty((2 * KS, 128, HC, 128), np.float32)
        sw13_p[0::2] = gb
        sw13_p[1::2] = ub
        m["sw13"] = np.ascontiguousarray(sw13_p)

        w2s = np.zeros((ISP, H), np.float32)
        w2s[:352] = sw2[:, lo:hi].T
        m["sw2"] = np.ascontiguousarray(
            w2s.reshape(KS, 128, HB, 512).transpose(2, 1, 0, 3)
        )
        in_maps.append(m)

    trace = bool(os.environ.get("MOE_BASS_TRACE"))
    if trace:
        _install_ntff_hook()
    res = None
    for attempt in range(3):
        try:
            res = run_bass_kernel_spmd(
                nc, in_maps, core_ids=list(range(N_CORES)), trace=trace
            )
            break
        except Exception:
            if attempt < 2:
                import time as _time

                _time.sleep(15)
    if res is None:
        # device unavailable/unrecoverable: exact (slow) host fallback
        return _host_moe(x, combine, w13, w2, sw13, sw2).reshape(
            hidden_states.shape
        )
    global LAST_EXEC_TIME_NS, LAST_TRACE
    LAST_EXEC_TIME_NS = res.exec_time_ns
    LAST_TRACE = res.instructions_and_trace

    out = np.zeros((T, H), np.float32)
    for core in range(N_CORES):
        out += res.results[core]["ys"]
        for s in range(2):
            e = 2 * core + s
            tok = ids[e]
            out[tok] += res.results[core][f"yout{s}"][: len(tok)]

    return out.reshape(hidden_states.shape).astype(np.float32)



# revision 3
# speedup vs baseline: 1.2677x; 1.2677x over previous
"""DeepseekV2 MoE (T=2048, H=2048, E=16 experts, top-6, I=1408, shared IS=2816)
on 8 Trainium2 NeuronCores.

Strategy (expert-parallel per the sharding hint), v2 (bf16):
  - Host: gate softmax/top-6 (numpy replica of the reference), per-expert
    token gather, weight transpose/packing into DMA-friendly bf16 layouts,
    and the final scatter/combine.
  - Device (SPMD, 8 cores), all matmuls bf16 (fp32 PSUM accumulation):
      * shared expert: 4x2 grid - core c owns token quarter (c//2) and
        intermediate half (c%2): mm1 = 22 exact 128-chunks, mm2 = 11 exact
        contraction chunks. Zero padding waste.
      * routed experts: experts sorted by routed-token count; slot0 = the
        8 largest (capacity cap0), slot1 = the 8 smallest (cap1), one of
        each per core -> per-core load is balanced by construction.
      * combine weights are folded into the u-side input on the host
        (xtc = x * combine_weight), so mm2 emits the already-scaled
        expert output and both mm1 and mm2 keep tokens as the moving
        operand (time scales with capacity, no 128-padding waste).
  - Outputs are stored transposed ([H-chunk, 128, tokens]) in bf16; the
    host transposes/accumulates in fp32. No collectives.
"""

import os
import numpy as np
import ml_dtypes

import concourse.bass as bass
import concourse.mybir as mybir
import concourse.tile as tile
from concourse.bass_utils import run_bass_kernel_spmd

F32 = mybir.dt.float32
BF16 = mybir.dt.bfloat16
AF = mybir.ActivationFunctionType
BF = ml_dtypes.bfloat16

# problem dims (hardcoded per spec)
T, H, I, E, TOP_K = 2048, 2048, 1408, 16, 6
FF = 2 * I              # 2816
IS = 2 * I              # shared intermediate (n_shared_experts=2 -> 2816)
N_CORES = 8
HC = H // 128           # 16 H chunks (contraction for mm1, output chunks mm2)
ICN = I // 128          # 11 I chunks (= shared half 1408/128 as well)
ST = T // 4             # 512 shared tokens per core (token quarter)


def _blocks(cap):
    """Moving-dim blocks: full-rate needs >=256; prefer 512 + tail."""
    if cap <= 512:
        return [(0, cap)]
    if cap - 512 < 256:
        h = ((cap + 63) // 128 + 1) // 2 * 64
        return [(0, h), (h, cap - h)]
    return [(0, 512), (512, cap - 512)]


def _split_excess_waits(nc, cap=1):
    """This container's walrus accepts at most one semaphore wait per
    instruction; move excess waits onto inserted same-engine NOPs."""
    for bb in nc.main_func.blocks:
        new_list = []
        for ins in bb.instructions:
            si = getattr(ins, "sync_info", None)
            waits = list(si.on_wait) if (si is not None and si.on_wait) else []
            if len(waits) > cap:
                excess, keep = waits[:-cap], waits[-cap:]
                si.on_wait = keep
                for i in range(0, len(excess), cap):
                    nop = mybir.InstNoOp(
                        name=f"I-waitsplit-{nc.next_id()}",
                        engine=ins.engine,
                        ins=[],
                        outs=[],
                        sync_info=mybir.SyncInfo(
                            on_update=[], on_wait=excess[i : i + cap]
                        ),
                        bass_nofuse=True,
                    )
                    nc.register_instruction(nop, overwrite=True)
                    new_list.append(nop)
            new_list.append(ins)
        bb.instructions = new_list


def build_nc(cap0: int, cap1: int):
    caps = (cap0, cap1)
    nc = bass.Bass()

    # --- DRAM parameters (packed bf16 layouts; partition dim first) ---
    xt_d = [
        nc.declare_dram_parameter(f"xt{s}", [128, HC, caps[s]], BF16, isOutput=False)
        for s in range(2)
    ]
    xtc_d = [
        nc.declare_dram_parameter(f"xtc{s}", [128, HC, caps[s]], BF16, isOutput=False)
        for s in range(2)
    ]
    # w13[e] chunks, order g0,u0,...,g10,u10: [22][128p(H), HC, 128f(F)]
    w13_d = [
        nc.declare_dram_parameter(f"w13_{s}", [2 * ICN, 128, HC, 128], BF16, isOutput=False)
        for s in range(2)
    ]
    # w2[e].T: [128p(I), ICN, HC, 128f(H)]
    w2_d = [
        nc.declare_dram_parameter(f"w2_{s}", [128, ICN, HC, 128], BF16, isOutput=False)
        for s in range(2)
    ]
    # shared: this core's token quarter / intermediate half
    xts_d = nc.declare_dram_parameter("xts", [128, HC, ST], BF16, isOutput=False)
    sw13_d = nc.declare_dram_parameter("sw13", [2 * ICN, 128, HC, 128], BF16, isOutput=False)
    sw2_d = nc.declare_dram_parameter("sw2", [128, ICN, HC, 128], BF16, isOutput=False)

    yt_d = [
        nc.declare_dram_parameter(f"yt{s}", [HC, 128, caps[s]], BF16, isOutput=True)
        for s in range(2)
    ]
    ys_d = nc.declare_dram_parameter("ys", [HC, 128, ST], BF16, isOutput=True)

    with tile.TileContext(nc) as tc:
        with (
            tc.tile_pool(name="xts", bufs=1) as p_xts,
            tc.tile_pool(name="xt", bufs=2) as p_xt,
            tc.tile_pool(name="w13", bufs=4) as p_w13,
            tc.tile_pool(name="w2", bufs=2) as p_w2,
            tc.tile_pool(name="aT", bufs=1) as p_aT,
            tc.tile_pool(name="tmp", bufs=3) as p_tmp,
            tc.tile_pool(name="y", bufs=3) as p_y,
            tc.tile_pool(name="ps", bufs=8, space="PSUM") as p_ps,
        ):
            def mm1(xt_sb, xtc_sb, w13_src, cap, aT_sb):
                """SwiGLU mm1 + silu*u: aT_sb[:, i, :] = silu(x@wg_i.T)*(xc@wu_i.T)
                (everything transposed: partition = F-chunk, free = tokens)."""
                blks = _blocks(cap)
                for i in range(ICN):
                    wg = p_w13.tile([128, HC, 128], BF16, tag="w13")
                    nc.sync.dma_start(out=wg[:], in_=w13_src[2 * i])
                    wu = p_w13.tile([128, HC, 128], BF16, tag="w13")
                    nc.sync.dma_start(out=wu[:], in_=w13_src[2 * i + 1])
                    for off, w in blks:
                        col = slice(off, off + w)
                        ps_g = p_ps.tile([128, 512], F32, tag="ps")
                        for hc in range(HC):
                            nc.tensor.matmul(
                                ps_g[:, :w], wg[:, hc, :], xt_sb[:, hc, col],
                                start=(hc == 0), stop=(hc == HC - 1),
                            )
                        ps_u = p_ps.tile([128, 512], F32, tag="ps")
                        for hc in range(HC):
                            nc.tensor.matmul(
                                ps_u[:, :w], wu[:, hc, :], xtc_sb[:, hc, col],
                                start=(hc == 0), stop=(hc == HC - 1),
                            )
                        tmp = p_tmp.tile([128, 512], BF16, tag="tmp")
                        nc.scalar.activation(
                            out=tmp[:, :w], in_=ps_g[:, :w], func=AF.Silu
                        )
                        nc.vector.tensor_mul(
                            out=aT_sb[:, i, col], in0=tmp[:, :w], in1=ps_u[:, :w]
                        )

            def mm2(aT_sb, w2_sb, cap, y_dst, evac_dve):
                """y.T[hchunk] = sum_i w2T[i,hchunk].T(stationary) @ aT[i](moving)."""
                blks = _blocks(cap)
                for hc in range(HC):
                    pss = []
                    for off, w in blks:
                        ps_y = p_ps.tile([128, 512], F32, tag="ps", name=f"psy{hc}_{off}")
                        pss.append(ps_y)
                    for i in range(ICN):
                        for b, (off, w) in enumerate(blks):
                            nc.tensor.matmul(
                                pss[b][:, :w],
                                w2_sb[:, i, hc, :],
                                aT_sb[:, i, off:off + w],
                                start=(i == 0), stop=(i == ICN - 1),
                            )
                    yst = p_y.tile([128, cap], BF16, tag="yst")
                    for b, (off, w) in enumerate(blks):
                        if evac_dve:
                            nc.vector.tensor_copy(yst[:, off:off + w], pss[b][:, :w])
                        else:
                            nc.scalar.activation(
                                out=yst[:, off:off + w], in_=pss[b][:, :w],
                                func=AF.Copy,
                            )
                    nc.gpsimd.dma_start(out=y_dst[hc], in_=yst[:])

            # ---- shared expert phase (first: streams per-hc so PE starts early)
            xts_sb = p_xts.tile([128, HC, ST], BF16, tag="xts")
            for hc in range(HC):
                nc.sync.dma_start(out=xts_sb[:, hc, :], in_=xts_d[:, hc, :])
            sw2_sb = p_w2.tile([128, ICN, HC, 128], BF16, tag="w2")
            nc.sync.dma_start(out=sw2_sb[:], in_=sw2_d[:])

            aT_s = p_aT.tile([128, ICN, max(ST, cap0, cap1)], BF16, tag="aT")
            mm1(xts_sb, xts_sb, sw13_d, ST, aT_s)
            mm2(aT_s, sw2_sb, ST, ys_d, evac_dve=True)

            # ---- routed expert phases
            for s in range(2):
                cap = caps[s]
                xt_sb = p_xt.tile([128, HC, cap0], BF16, tag="xt")
                nc.sync.dma_start(out=xt_sb[:, :, :cap], in_=xt_d[s][:])
                xtc_sb = p_xt.tile([128, HC, cap0], BF16, tag="xt")
                nc.sync.dma_start(out=xtc_sb[:, :, :cap], in_=xtc_d[s][:])
                w2_sb = p_w2.tile([128, ICN, HC, 128], BF16, tag="w2")
                nc.sync.dma_start(out=w2_sb[:], in_=w2_d[s][:])

                aT = p_aT.tile([128, ICN, max(ST, cap0, cap1)], BF16, tag="aT")
                mm1(xt_sb[:, :, :cap], xtc_sb[:, :, :cap], w13_d[s], cap, aT)
                mm2(aT, w2_sb, cap, yt_d[s], evac_dve=False)

    _split_excess_waits(nc, cap=1)
    return nc


# ------------------------- host side -------------------------

def _gate_combine(x, gate_w):
    """Replica of the reference gate in pure numpy (f32). The top-6 selection
    is rounding-robust (min rank-6/7 logit gap over tokens ~7e-5 vs ~1e-6
    cross-implementation noise). Ties break like lax.top_k (lowest index)."""
    z = (x @ gate_w.T).astype(np.float32)                 # [T, E] logits
    z64 = z.astype(np.float64)
    m = z64.max(-1, keepdims=True)
    ez = np.exp(z64 - m)
    scores = (ez / ez.sum(-1, keepdims=True)).astype(np.float32)
    order = np.argsort(-scores, axis=-1, kind="stable")[:, :TOP_K]
    topk_w = np.take_along_axis(scores, order, axis=-1)
    topk_w = topk_w / (topk_w.sum(-1, keepdims=True) + 1e-20)
    combine = np.zeros((x.shape[0], E), np.float32)
    np.put_along_axis(combine, order, topk_w, axis=-1)
    return combine


def _pack_xT(xTcols, cap):
    """xTcols [H, n<=cap] f32 -> [128, HC, cap] bf16 (zero-padded)."""
    out = np.zeros((128, HC, cap), BF)
    n = xTcols.shape[1]
    out[:, :, :n] = xTcols.reshape(HC, 128, n).transpose(1, 0, 2).astype(BF)
    return out


def _pack_w13(w13e):
    """[FF, H] f32 -> [22, 128, HC, 128] bf16, order g0,u0,g1,u1,..."""
    a = w13e.reshape(2 * ICN, 128, HC, 128).transpose(0, 3, 2, 1)
    order = np.empty(2 * ICN, np.int64)
    order[0::2] = np.arange(ICN)
    order[1::2] = np.arange(ICN) + ICN
    return np.ascontiguousarray(a[order]).astype(BF)


def _pack_w2T(w2e):
    """[H, I'] f32 -> [128, I'/128, HC, 128] bf16 (w2T[i, h] layout)."""
    icn = w2e.shape[1] // 128
    return np.ascontiguousarray(
        w2e.reshape(HC, 128, icn, 128).transpose(3, 2, 0, 1)
    ).astype(BF)


def _host_moe(x, combine, w13, w2, sw13, sw2):
    """Exact numpy fallback (only used on absurd routing imbalance)."""

    def silu(v):
        return v / (1.0 + np.exp(-v))

    out = np.zeros((T, H), np.float32)
    for e in range(E):
        gu = x @ w13[e].T
        a = silu(gu[:, :I]) * gu[:, I:]
        out += combine[:, e:e + 1] * (a @ w2[e].T)
    gu = x @ sw13.T
    a = silu(gu[:, :IS]) * gu[:, IS:]
    out += a @ sw2.T
    return out


_NC_CACHE = {}

LAST_EXEC_TIME_NS = None
LAST_TRACE = None


def _install_ntff_hook():
    """Bridge the missing ``antenv.axon_hooks`` module so trace=True works
    in this container (used by test.py only; harmless if already present)."""
    import sys, types

    try:
        from antenv.axon_hooks import get_axon_ntff_profile_hook  # noqa: F401
        return
    except ImportError:
        pass
    import antenv  # noqa: F401
    import trn_agent_boot.trn_boot as tb

    mod = types.ModuleType("antenv.axon_hooks")
    _h = [None]
    mod.set_axon_ntff_profile_hook = lambda h: _h.__setitem__(0, h)
    mod.get_axon_ntff_profile_hook = lambda: _h[0]
    sys.modules["antenv.axon_hooks"] = mod
    mod.set_axon_ntff_profile_hook(
        tb._ntff_profile_via_ctypes("/opt/axon/libaxon_pjrt.so")
    )


def kernel(hidden_states, gate_w, w13, w2, sw13, sw2):
    hidden_states = np.asarray(hidden_states)
    x = np.ascontiguousarray(hidden_states.reshape(T, H), dtype=np.float32)
    gate_w = np.asarray(gate_w, dtype=np.float32)
    w13 = np.asarray(w13, dtype=np.float32)
    w2 = np.asarray(w2, dtype=np.float32)
    sw13 = np.asarray(sw13, dtype=np.float32)
    sw2 = np.asarray(sw2, dtype=np.float32)

    combine = _gate_combine(x, gate_w)          # [T, E]

    ids = [np.nonzero(combine[:, e] > 0)[0] for e in range(E)]
    cnt = np.array([len(i) for i in ids])
    order = np.argsort(-cnt, kind="stable")
    top8, bot8 = order[:8], order[8:]

    def r32(v):
        return max(64, int(-(-v // 32) * 32))

    cap0 = r32(cnt[top8].max())
    cap1 = r32(max(1, cnt[bot8].max()))
    if cap0 > T:
        # Essentially impossible for randn-style inputs; exact host fallback.
        return _host_moe(x, combine, w13, w2, sw13, sw2).reshape(
            hidden_states.shape
        )

    if (cap0, cap1) not in _NC_CACHE:
        _NC_CACHE[(cap0, cap1)] = build_nc(cap0, cap1)
    nc = _NC_CACHE[(cap0, cap1)]

    xT = np.ascontiguousarray(x.T)              # [H, T] f32

    in_maps = []
    for core in range(N_CORES):
        m = {}
        # routed slots
        for s, (elist, cap) in enumerate(((top8, cap0), (bot8, cap1))):
            e = int(elist[core])
            tok = ids[e]
            cols = xT[:, tok]
            m[f"xt{s}"] = _pack_xT(cols, cap)
            m[f"xtc{s}"] = _pack_xT(cols * combine[tok, e][None, :], cap)
            m[f"w13_{s}"] = _pack_w13(w13[e])
            m[f"w2_{s}"] = _pack_w2T(w2[e])
        # shared expert: token quarter q, intermediate half h
        q, hh = core // 2, core % 2
        m["xts"] = _pack_xT(xT[:, q * ST:(q + 1) * ST], ST)
        lo = hh * I
        g = sw13[lo:lo + I]
        u = sw13[IS + lo:IS + lo + I]
        sw13_p = np.empty((2 * ICN, 128, HC, 128), BF)
        sw13_p[0::2] = g.reshape(ICN, 128, HC, 128).transpose(0, 3, 2, 1).astype(BF)
        sw13_p[1::2] = u.reshape(ICN, 128, HC, 128).transpose(0, 3, 2, 1).astype(BF)
        m["sw13"] = np.ascontiguousarray(sw13_p)
        m["sw2"] = _pack_w2T(sw2[:, lo:lo + I])
        in_maps.append(m)

    trace = bool(os.environ.get("MOE_BASS_TRACE"))
    if trace:
        _install_ntff_hook()
    res = None
    for attempt in range(3):
        try:
            res = run_bass_kernel_spmd(
                nc, in_maps, core_ids=list(range(N_CORES)), trace=trace
            )
            break
        except Exception:
            if attempt < 2:
                import time as _time

                _time.sleep(15)
    if res is None:
        # device unavailable/unrecoverable: exact (slow) host fallback
        return _host_moe(x, combine, w13, w2, sw13, sw2).reshape(
            hidden_states.shape
        )
    global LAST_EXEC_TIME_NS, LAST_TRACE
    LAST_EXEC_TIME_NS = res.exec_time_ns
    LAST_TRACE = res.instructions_and_trace

    out = np.zeros((T, H), np.float32)
    for core in range(N_CORES):
        q = core // 2
        ys = res.results[core]["ys"].astype(np.float32)     # [HC, 128, ST]
        out[q * ST:(q + 1) * ST] += ys.transpose(2, 0, 1).reshape(ST, H)
        for s, elist in enumerate((top8, bot8)):
            e = int(elist[core])
            tok = ids[e]
            yt = res.results[core][f"yt{s}"].astype(np.float32)  # [HC,128,cap]
            yt = yt.transpose(2, 0, 1).reshape(-1, H)
            out[tok] += yt[: len(tok)]

    return out.reshape(hidden_states.shape).astype(np.float32)
